# revision 24
# baseline (speedup 1.0000x reference)
"""Multi-head attention block (QKV proj -> softmax attention -> out proj) for
Trainium2, SPMD across 8 NeuronCores.

Sharding: batch (B=2) x head-groups (4 groups of 4 heads). Core c handles
batch c//4 and heads [4*(c%4), 4*(c%4)+4). Each core computes its partial
output contribution (context @ wo_slice.T); the host sums the 4 head-group
partials per batch (tensor-parallel row-sharded wo => the all-reduce is the
host-side gather).

All matmuls run in bf16 with fp32 PSUM accumulation. Softmax runs in fp32
out of PSUM (exp on the scalar engine); softmax denominators are built on
the DVE (pairwise tree-sum of the exp tiles, then a DMA x-bar transpose
plus a free-axis reduce) so the tensor engine only executes real matmuls.

Scheduling notes:
 - the q projection for the last token block is deferred into the first
   attention units (which otherwise idle while the scalar engine runs exp)
 - out-projection groups are pulled from a global FIFO, gated on the
   producing normalization being emitted >= 1 unit earlier, so the PE
   queue never head-of-line blocks on a just-written context tile
 - PV matmuls trail the exp by two score pairs so they never wait on ACT
"""

import sys

if "/opt/trn_rl_repo" not in sys.path:
    sys.path.insert(0, "/opt/trn_rl_repo")

from contextlib import ExitStack

import ml_dtypes
import numpy as np

import concourse.bacc as bacc
import concourse.tile as tile
from concourse import mybir
from concourse.bass_utils import run_bass_kernel_spmd

BF16 = mybir.dt.bfloat16
F32 = mybir.dt.float32

B, S, DIM = 2, 2048, 2048
HEADS, HD = 16, 128
P = 128
N_CORES = 8
HGROUPS = 4  # head groups (second shard axis is batch)
HPC = HEADS // HGROUPS  # heads per core = 4
DL = HPC * HD  # local head dims per core = 512
SCALE = 1.0 / float(np.sqrt(HD))

NK = DIM // P  # 16 contraction tiles for the projections
NM = S // 512  # 4 m-chunks (tokens)
NQ = S // P  # 16 q tiles
NN = S // P  # 16 kv tiles
NE = DIM // 512  # 4 output-dim chunks

_PROGRAM_CACHE = {}


def _emit(nc, tc, xT, wqT, wkT, wvT, woT, maskf, out):
    with_mask = maskf is not None
    with ExitStack() as octx:
        planes = octx.enter_context(tc.tile_pool(name="planes", bufs=1))
        q_sb = [planes.tile([P, S], BF16, tag=f"q{h}", name=f"q{h}") for h in range(HPC)]
        k_sb = [planes.tile([P, S], BF16, tag=f"k{h}", name=f"k{h}") for h in range(HPC)]
        ctx_sb = [planes.tile([P, S], BF16, tag=f"ctx{h}", name=f"ctx{h}") for h in range(HPC)]

        vv_pool = octx.enter_context(tc.tile_pool(name="vv", bufs=1))
        vvs = [vv_pool.tile([P, NN, P], BF16, tag=f"vv{h}", name=f"vv{h}")
               for h in range(HPC)]

        # q weights + the last m-chunk's x tiles survive into phase 2 (the
        # q(mc=3) projection groups run interleaved into the first units)
        wqpool = octx.enter_context(tc.tile_pool(name="wqp", bufs=1))
        wq_sb = wqpool.tile([P, NK * DL], BF16, tag="wq", name="wq_sb")
        xpin_pool = octx.enter_context(tc.tile_pool(name="xpin", bufs=1))
        xpin = [xpin_pool.tile([P, 512], BF16, tag=f"xp{kt}", name=f"xp{kt}")
                for kt in range(NK)]

        # ---------------- Phase 1: QKV projections ----------------
        with ExitStack() as ctx:
            wpool = ctx.enter_context(tc.tile_pool(name="wqkv", bufs=1))
            vT_sb = [wpool.tile([P, S], BF16, tag=f"vt{h}", name=f"vt{h}")
                     for h in range(HPC)]
            w_sb = {"q": wq_sb}
            for name in ("k", "v"):
                w_sb[name] = wpool.tile([P, NK * DL], BF16, tag=f"w{name}",
                                        name=f"w{name}")
            # weights stream on the scalar+gpsimd DMA queues in consumption
            # order (q split across both, then k / v) so the first projection
            # group is gated only by q+x arrival; x keeps the sync queue
            for kt in range(NK):
                eng = nc.scalar if kt % 2 == 0 else nc.gpsimd
                eng.dma_start(
                    w_sb["q"][:, kt * DL : (kt + 1) * DL],
                    wqT[kt * P : (kt + 1) * P, :],
                )
            for name, srct, eng in (("k", wkT, nc.scalar), ("v", wvT, nc.gpsimd)):
                for kt in range(NK):
                    eng.dma_start(
                        w_sb[name][:, kt * DL : (kt + 1) * DL],
                        srct[kt * P : (kt + 1) * P, :],
                    )
            xpool = ctx.enter_context(tc.tile_pool(name="xt", bufs=2 * NK))
            pq = ctx.enter_context(tc.tile_pool(name="ps_qkv", bufs=4, space="PSUM"))

            for mc in range(NM):
                if mc == NM - 1:
                    xts = xpin
                else:
                    xts = [xpool.tile([P, 512], BF16, tag="xt", name="xt")
                           for kt in range(NK)]
                for kt in range(NK):
                    nc.sync.dma_start(
                        xts[kt][:],
                        xT[kt * P : (kt + 1) * P, mc * 512 : (mc + 1) * 512],
                    )
                # last m-chunk: only k/v here (v first so the vv transposes
                # start early); its q groups run inside the first units
                if mc == NM - 1:
                    name_order = (("v", vT_sb), ("k", k_sb))
                else:
                    name_order = (("q", q_sb), ("k", k_sb), ("v", vT_sb))
                for name, plane_list in name_order:
                    for h in range(HPC):
                        ps = pq.tile([P, 512], F32, tag="ps")
                        for kt in range(NK):
                            nc.tensor.matmul(
                                ps[:],
                                w_sb[name][:, kt * DL + h * P : kt * DL + (h + 1) * P],
                                xts[kt][:],
                                start=(kt == 0),
                                stop=(kt == NK - 1),
                            )
                        nc.vector.tensor_copy(
                            plane_list[h][:, mc * 512 : (mc + 1) * 512], ps[:]
                        )
                        if name == "v" and mc == NM - 1:
                            nc.sync.dma_start(
                                vvs[h][:], vT_sb[h][:], transpose=True
                            )

        # ------- Phase 2+3: attention (scoresT form) + out projection -------
        with ExitStack() as ctx:
            wopool = ctx.enter_context(tc.tile_pool(name="wo", bufs=1))
            wo_sb = [wopool.tile([P, DIM], BF16, tag=f"wo{h}", name=f"wo{h}")
                     for h in range(HPC)]
            for h in range(HPC):
                nc.gpsimd.dma_start(wo_sb[h][:], woT[h * P : (h + 1) * P, :])

            pbt_pool = ctx.enter_context(tc.tile_pool(name="pbt", bufs=3))
            atree = ctx.enter_context(tc.tile_pool(name="atree", bufs=2))
            pst_pool = ctx.enter_context(tc.tile_pool(name="pst", bufs=3))
            ctmp_pool = ctx.enter_context(tc.tile_pool(name="ctmp", bufs=3))
            cnrm_pool = ctx.enter_context(tc.tile_pool(name="cnrm", bufs=3))
            stats = ctx.enter_context(tc.tile_pool(name="stats", bufs=4))
            opool = ctx.enter_context(tc.tile_pool(name="ob", bufs=3))
            if with_mask:
                mpool = ctx.enter_context(tc.tile_pool(name="mask", bufs=4))
            ps_s = ctx.enter_context(tc.tile_pool(name="ps_s", bufs=3, space="PSUM"))
            # separate pools: the PV accumulator must never wait on a
            # d_group's out-copy to free a shared ring buffer
            ps_cd = ctx.enter_context(tc.tile_pool(name="ps_cd", bufs=1, space="PSUM"))
            ps_dg = ctx.enter_context(tc.tile_pool(name="ps_dg", bufs=1, space="PSUM"))

            # ---- interleavable PE work: deferred q(mc3) + out-proj FIFO ----
            DG = [(tt, ec) for tt in range(4 * (NM - 1)) for ec in range(NE)]
            dstate = {"taken": 0, "avail": 0}
            q3 = {"i": 0, "ps": None}
            last_mc = NM - 1

            def q3_item():
                """One quarter (4 kt) of a deferred q(mc3, h) group."""
                it = q3["i"]
                h, qu = it // 4, it % 4
                q3["i"] += 1
                if qu == 0:
                    q3["ps"] = ps_dg.tile([P, 512], F32, tag="ps_dg",
                                          name="q3ps")
                ps = q3["ps"]
                for kt in range(4 * qu, 4 * qu + 4):
                    nc.tensor.matmul(
                        ps[:],
                        wq_sb[:, kt * DL + h * P : kt * DL + (h + 1) * P],
                        xpin[kt][:],
                        start=(kt == 0),
                        stop=(kt == NK - 1),
                    )
                if qu == 3:
                    nc.any.tensor_copy(
                        q_sb[h][:, last_mc * 512 : (last_mc + 1) * 512], ps[:]
                    )
                    q3["ps"] = None

            def d_group(tt, ec):
                ps = ps_dg.tile([P, 512], F32, tag="ps_dg")
                for h in range(HPC):
                    nc.tensor.matmul(
                        ps[:],
                        ctx_sb[h][:, tt * P : (tt + 1) * P],
                        wo_sb[h][:, ec * 512 : (ec + 1) * 512],
                        start=(h == 0),
                        stop=(h == HPC - 1),
                    )
                ob = opool.tile([P, 512], F32, tag="ob")
                nc.vector.tensor_copy(ob[:], ps[:])
                nc.gpsimd.dma_start(
                    out[tt * P : (tt + 1) * P, ec * 512 : (ec + 1) * 512], ob[:]
                )

            def unit(jb, h, cap):
                """One (q-block, head): scoresT+exp, PV (2-pair lag), DVE
                denominator sums, deferred normalization; pulls up to `cap`
                interleaved PE work items (q3 quarters / out-proj groups)."""
                pulls = [0]

                def take():
                    if pulls[0] >= cap:
                        return
                    if q3["i"] < 4 * HPC:
                        q3_item()
                        pulls[0] += 1
                    elif dstate["taken"] < dstate["avail"]:
                        d_group(*DG[dstate["taken"]])
                        dstate["taken"] += 1
                        pulls[0] += 1

                qs = q_sb[h][:, jb * 512 : (jb + 1) * 512]
                pbt = pbt_pool.tile([P, NN, 512], BF16, tag="pbt", name="pbt")
                pbt_flat = pbt[:].rearrange("p n m -> p (n m)")
                psc = ps_cd.tile([P, 512], F32, tag="ps_cd")
                # wide-op denominator tree: A[q] = quarter partial [P,1024]
                aq = [atree.tile([P, 1024], BF16, tag=f"a{i}", name=f"a{i}")
                      for i in range(4)]

                def pv(nt):
                    nc.tensor.matmul(
                        psc[:], vvs[h][:, nt, :], pbt[:, nt, :],
                        start=(nt == 0), stop=(nt == NN - 1),
                    )

                for pair in range(8):
                    take()
                    ps = ps_s.tile([P, 1024], F32, tag="ps_s")
                    for sub in range(2):
                        nt = 2 * pair + sub
                        nc.tensor.matmul(
                            ps[:, sub * 512 : (sub + 1) * 512],
                            k_sb[h][:, nt * P : (nt + 1) * P],
                            qs,
                            start=True,
                            stop=True,
                        )
                    if with_mask:
                        mt = mpool.tile([P, 1024], F32, tag="mt")
                        for sub in range(2):
                            nt = 2 * pair + sub
                            nc.gpsimd.dma_start(
                                mt[:, sub * 512 : (sub + 1) * 512],
                                maskf[nt * P : (nt + 1) * P,
                                      jb * 512 : (jb + 1) * 512],
                            )
                        nc.vector.tensor_add(ps[:], ps[:], mt[:])
                    nc.scalar.activation(
                        pbt_flat[:, pair * 1024 : (pair + 1) * 1024],
                        ps[:],
                        mybir.ActivationFunctionType.Exp,
                        scale=SCALE,
                    )
                    # denominator partials on the DVE: after each odd pair a
                    # quarter (4 exp tiles) is complete -> one 1024-wide add
                    if pair % 2 == 1:
                        qq = pair // 2
                        nc.vector.tensor_add(
                            aq[qq][:],
                            pbt_flat[:, 4 * qq * 512 : (4 * qq + 2) * 512],
                            pbt_flat[:, (4 * qq + 2) * 512 : (4 * qq + 4) * 512],
                        )
                        if qq == 1:
                            nc.vector.tensor_add(aq[0][:], aq[0][:], aq[1][:])
                    # PV trails the exp by two pairs so it never waits on ACT
                    if pair >= 2:
                        pv(2 * (pair - 2))
                        pv(2 * (pair - 2) + 1)
                for nt in range(12, 16):
                    pv(nt)
                nc.vector.tensor_add(aq[2][:], aq[2][:], aq[3][:])
                nc.vector.tensor_add(aq[0][:], aq[0][:], aq[2][:])

                ctmp = ctmp_pool.tile([P, 512], BF16, tag="ctmp", name="ctmp")
                nc.scalar.activation(
                    ctmp[:], psc[:], mybir.ActivationFunctionType.Copy
                )

                def finish():
                    # denominator: x-bar transpose of the [P,1024] half-sum
                    # -> free-axis reduce -> fold halves -> wide reciprocal
                    pst = pst_pool.tile([P, 8, P], BF16, tag="pst", name="pst")
                    nc.sync.dma_start(pst[:], aq[0][:], transpose=True)
                    den8 = stats.tile([P, 8], F32, tag="den8")
                    nc.vector.tensor_reduce(
                        den8[:], pst[:], mybir.AxisListType.X, mybir.AluOpType.add
                    )
                    den = stats.tile([P, 4], F32, tag="den")
                    nc.vector.tensor_add(den[:], den8[:, 0:4], den8[:, 4:8])
                    rect = stats.tile([P, 4], F32, tag="rect")
                    nc.vector.reciprocal(rect[:], den[:])
                    cn = cnrm_pool.tile([P, 4, P], BF16, tag="cnrm", name="cnrm")
                    nc.sync.dma_start(cn[:], ctmp[:], transpose=True)
                    # normalization multiplies on the (idle) gpsimd engine
                    for j in range(4):
                        nc.gpsimd.tensor_scalar_mul(
                            cn[:, j, :], cn[:, j, :], rect[:, j : j + 1]
                        )
                    ctx_dst = ctx_sb[h][:, jb * 512 : (jb + 1) * 512].rearrange(
                        "p (a b) -> p a b", a=4
                    )
                    nc.sync.dma_start(ctx_dst, cn[:], transpose=True)

                return finish

            pending = []  # (finish, avail_bump)
            for jb in range(4):
                for h in range(HPC):
                    cap = 6 if jb == 3 else 4
                    fin = unit(jb, h, cap)
                    pending.append((fin, 16 if h == HPC - 1 else 0))
                    if len(pending) > 1:
                        f, bump = pending.pop(0)
                        f()
                        dstate["avail"] += bump
            for f, bump in pending:
                f()
                dstate["avail"] += bump
            while dstate["taken"] < len(DG):
                d_group(*DG[dstate["taken"]])
                dstate["taken"] += 1
            for tt in range(4 * (NM - 1), NQ):
                for ec in range(NE):
                    d_group(tt, ec)


def _build(with_mask: bool):
    nc = bacc.Bacc("TRN2")
    xT = nc.dram_tensor("xT", [DIM, S], BF16, kind="ExternalInput")
    wqT = nc.dram_tensor("wqT", [DIM, DL], BF16, kind="ExternalInput")
    wkT = nc.dram_tensor("wkT", [DIM, DL], BF16, kind="ExternalInput")
    wvT = nc.dram_tensor("wvT", [DIM, DL], BF16, kind="ExternalInput")
    woT = nc.dram_tensor("woT", [DL, DIM], BF16, kind="ExternalInput")
    maskf = (
        nc.dram_tensor("maskf", [S, S], F32, kind="ExternalInput")
        if with_mask
        else None
    )
    out = nc.dram_tensor("out", [S, DIM], F32, kind="ExternalOutput")
    with tile.TileContext(nc) as tc:
        _emit(nc, tc, xT, wqT, wkT, wvT, woT, maskf, out)
    nc.finalize()
    return nc


def _get_program(with_mask: bool):
    if with_mask not in _PROGRAM_CACHE:
        _PROGRAM_CACHE[with_mask] = _build(with_mask)
    return _PROGRAM_CACHE[with_mask]


def _prep_in_maps(x, mask, wq, wk, wv, wo, with_mask):
    bf = ml_dtypes.bfloat16
    f32 = np.float32
    xTs = [np.ascontiguousarray(x[b].T.astype(bf)) for b in range(B)]
    if with_mask:
        maskf = np.ascontiguousarray(mask[0, 0].T.astype(f32) / SCALE)
    in_maps = []
    for c in range(N_CORES):
        b = c // HGROUPS
        g = c % HGROUPS
        sl = slice(g * DL, (g + 1) * DL)
        m = {
            "xT": xTs[b],
            "wqT": np.ascontiguousarray(wq[sl, :].T.astype(bf)),
            "wkT": np.ascontiguousarray(wk[sl, :].T.astype(bf)),
            "wvT": np.ascontiguousarray(wv[sl, :].T.astype(bf)),
            "woT": np.ascontiguousarray(wo[:, sl].T.astype(bf)),
        }
        if with_mask:
            m["maskf"] = maskf
        in_maps.append(m)
    return in_maps


def run_sharded(x, mask, wq, wk, wv, wo, trace=False, trace_kwargs=None):
    """Run the SPMD kernel; returns (full_output, BassKernelResults)."""
    with_mask = bool(np.any(np.asarray(mask)))
    nc = _get_program(with_mask)
    in_maps = _prep_in_maps(
        np.asarray(x), np.asarray(mask), np.asarray(wq), np.asarray(wk),
        np.asarray(wv), np.asarray(wo), with_mask,
    )
    kw = {}
    if trace:
        kw["trace"] = True
        if trace_kwargs:
            kw["trace_kwargs"] = trace_kwargs
    res = run_bass_kernel_spmd(nc, in_maps, list(range(N_CORES)), **kw)
    out = np.zeros((B, S, DIM), np.float32)
    for c in range(N_CORES):
        out[c // HGROUPS] += res.results[c]["out"]
    return out, res


def kernel(**inputs):
    out, _ = run_sharded(
        inputs["x"], inputs["mask"], inputs["wq"], inputs["wk"], inputs["wv"],
        inputs["wo"],
    )
    return out


# revision 33
# speedup vs baseline: 1.2985x; 1.2985x over previous
"""Multi-head attention block (QKV proj -> softmax attention -> out proj) for
Trainium2, SPMD across 8 NeuronCores.

Sharding: batch (B=2) x head-groups (4 groups of 4 heads). Core c handles
batch c//4 and heads [4*(c%4), 4*(c%4)+4). Each core computes its partial
output contribution (context @ wo_slice.T); the host sums the 4 head-group
partials per batch (tensor-parallel row-sharded wo => the all-reduce is the
host-side gather).

All matmuls run in bf16 with fp32 PSUM accumulation. Softmax runs in fp32
out of PSUM (exp on the scalar engine); softmax denominators are built on
the DVE (pairwise tree-sum of the exp tiles, then a DMA x-bar transpose
plus a free-axis reduce) so the tensor engine only executes real matmuls.

Scheduling notes:
 - the q projection for the last token block is deferred into the first
   attention units (which otherwise idle while the scalar engine runs exp)
 - out-projection groups are pulled from a global FIFO, gated on the
   producing normalization being emitted >= 1 unit earlier, so the PE
   queue never head-of-line blocks on a just-written context tile
 - PV matmuls trail the exp by two score pairs so they never wait on ACT
"""

import sys

if "/opt/trn_rl_repo" not in sys.path:
    sys.path.insert(0, "/opt/trn_rl_repo")

from contextlib import ExitStack

import ml_dtypes
import numpy as np

import concourse.bacc as bacc
import concourse.tile as tile
from concourse import mybir
from concourse.bass_utils import run_bass_kernel_spmd

BF16 = mybir.dt.bfloat16
F32 = mybir.dt.float32

B, S, DIM = 2, 2048, 2048
HEADS, HD = 16, 128
P = 128
N_CORES = 8
HGROUPS = 4  # head groups (second shard axis is batch)
HPC = HEADS // HGROUPS  # heads per core = 4
DL = HPC * HD  # local head dims per core = 512
SCALE = 1.0 / float(np.sqrt(HD))

NK = DIM // P  # 16 contraction tiles for the projections
NM = S // 512  # 4 m-chunks (tokens)
NQ = S // P  # 16 q tiles
NN = S // P  # 16 kv tiles
NE = DIM // 512  # 4 output-dim chunks

_PROGRAM_CACHE = {}


def _emit(nc, tc, xT, wqT, wkT, wvT, woT, maskf, out):
    with_mask = maskf is not None
    with ExitStack() as octx:
        planes = octx.enter_context(tc.tile_pool(name="planes", bufs=1))
        q_sb = [planes.tile([P, S], BF16, tag=f"q{h}", name=f"q{h}") for h in range(HPC)]
        k_sb = [planes.tile([P, S], BF16, tag=f"k{h}", name=f"k{h}") for h in range(HPC)]
        ctx_sb = [planes.tile([P, S], BF16, tag=f"ctx{h}", name=f"ctx{h}") for h in range(HPC)]

        vv_pool = octx.enter_context(tc.tile_pool(name="vv", bufs=1))
        vvs = [vv_pool.tile([P, NN, P], BF16, tag=f"vv{h}", name=f"vv{h}")
               for h in range(HPC)]

        # q weights + the last m-chunk's x tiles survive into phase 2 (the
        # q(mc=3) projection groups run interleaved into the first units)
        wqpool = octx.enter_context(tc.tile_pool(name="wqp", bufs=1))
        wq_sb = wqpool.tile([P, NK * DL], BF16, tag="wq", name="wq_sb")
        xpin_pool = octx.enter_context(tc.tile_pool(name="xpin", bufs=1))
        xpin = [xpin_pool.tile([P, 512], BF16, tag=f"xp{kt}", name=f"xp{kt}")
                for kt in range(NK)]

        # ---------------- Phase 1: QKV projections ----------------
        with ExitStack() as ctx:
            wpool = ctx.enter_context(tc.tile_pool(name="wqkv", bufs=1))
            vT_sb = [wpool.tile([P, S], BF16, tag=f"vt{h}", name=f"vt{h}")
                     for h in range(HPC)]
            w_sb = {"q": wq_sb}
            for name in ("k", "v"):
                w_sb[name] = wpool.tile([P, NK * DL], BF16, tag=f"w{name}",
                                        name=f"w{name}")
            # weights stream on the scalar+gpsimd DMA queues in consumption
            # order (q split across both, then k / v) so the first projection
            # group is gated only by q+x arrival; x keeps the sync queue
            for kt in range(NK):
                eng = nc.scalar if kt % 2 == 0 else nc.gpsimd
                eng.dma_start(
                    w_sb["q"][:, kt * DL : (kt + 1) * DL],
                    wqT[kt * P : (kt + 1) * P, :],
                )
            for name, srct, eng in (("k", wkT, nc.scalar), ("v", wvT, nc.gpsimd)):
                for kt in range(NK):
                    eng.dma_start(
                        w_sb[name][:, kt * DL : (kt + 1) * DL],
                        srct[kt * P : (kt + 1) * P, :],
                    )
            xpool = ctx.enter_context(tc.tile_pool(name="xt", bufs=2 * NK))
            pq = ctx.enter_context(tc.tile_pool(name="ps_qkv", bufs=4, space="PSUM"))

            for mc in range(NM):
                if mc == NM - 1:
                    xts = xpin
                else:
                    xts = [xpool.tile([P, 512], BF16, tag="xt", name="xt")
                           for kt in range(NK)]
                for kt in range(NK):
                    nc.sync.dma_start(
                        xts[kt][:],
                        xT[kt * P : (kt + 1) * P, mc * 512 : (mc + 1) * 512],
                    )
                # last m-chunk: only k/v here (v first so the vv transposes
                # start early); its q groups run inside the first units
                if mc == NM - 1:
                    name_order = (("v", vT_sb), ("k", k_sb))
                else:
                    name_order = (("q", q_sb), ("k", k_sb), ("v", vT_sb))
                for name, plane_list in name_order:
                    for h in range(HPC):
                        ps = pq.tile([P, 512], F32, tag="ps")
                        for kt in range(NK):
                            nc.tensor.matmul(
                                ps[:],
                                w_sb[name][:, kt * DL + h * P : kt * DL + (h + 1) * P],
                                xts[kt][:],
                                start=(kt == 0),
                                stop=(kt == NK - 1),
                            )
                        nc.vector.tensor_copy(
                            plane_list[h][:, mc * 512 : (mc + 1) * 512], ps[:]
                        )
                        if name == "v" and mc == NM - 1:
                            nc.sync.dma_start(
                                vvs[h][:], vT_sb[h][:], transpose=True
                            )

        # ------- Phase 2+3: attention (scoresT form) + out projection -------
        with ExitStack() as ctx:
            wopool = ctx.enter_context(tc.tile_pool(name="wo", bufs=1))
            wo_sb = [wopool.tile([P, DIM], BF16, tag=f"wo{h}", name=f"wo{h}")
                     for h in range(HPC)]
            for h in range(HPC):
                nc.gpsimd.dma_start(wo_sb[h][:], woT[h * P : (h + 1) * P, :])

            pbt_pool = ctx.enter_context(tc.tile_pool(name="pbt", bufs=3))
            atree = ctx.enter_context(tc.tile_pool(name="atree", bufs=2))
            qf_pool = ctx.enter_context(tc.tile_pool(name="qf", bufs=3))
            pst_pool = ctx.enter_context(tc.tile_pool(name="pst", bufs=3))
            ctmp_pool = ctx.enter_context(tc.tile_pool(name="ctmp", bufs=3))
            cnrm_pool = ctx.enter_context(tc.tile_pool(name="cnrm", bufs=3))
            stats = ctx.enter_context(tc.tile_pool(name="stats", bufs=4))
            opool = ctx.enter_context(tc.tile_pool(name="ob", bufs=3))
            if with_mask:
                mpool = ctx.enter_context(tc.tile_pool(name="mask", bufs=4))
            ps_s = ctx.enter_context(tc.tile_pool(name="ps_s", bufs=2, space="PSUM"))
            # separate pools: the PV accumulator must never wait on a
            # d_group's out-copy to free a shared ring buffer
            ps_cd = ctx.enter_context(tc.tile_pool(name="ps_cd", bufs=2, space="PSUM"))
            ps_dg = ctx.enter_context(tc.tile_pool(name="ps_dg", bufs=1, space="PSUM"))

            # ---- interleavable PE work: deferred q(mc3) + out-proj FIFO ----
            # out-projection in double-width groups: [P,1024] PSUM spanning
            # two output chunks -> one wide out-copy on the DVE
            DG = [(tt, eh) for tt in range(4 * (NM - 1)) for eh in range(NE // 2)]
            dstate = {"taken": 0, "avail": 0}
            q3 = {"i": 0, "ps": None}
            last_mc = NM - 1

            def q3_item():
                """One quarter (4 kt) of a deferred q(mc3, h) group."""
                it = q3["i"]
                h, qu = it // 4, it % 4
                q3["i"] += 1
                if qu == 0:
                    q3["ps"] = ps_cd.tile([P, 512], F32, tag="ps_cd",
                                          name="q3ps")
                ps = q3["ps"]
                for kt in range(4 * qu, 4 * qu + 4):
                    nc.tensor.matmul(
                        ps[:],
                        wq_sb[:, kt * DL + h * P : kt * DL + (h + 1) * P],
                        xpin[kt][:],
                        start=(kt == 0),
                        stop=(kt == NK - 1),
                    )
                if qu == 3:
                    nc.vector.tensor_copy(
                        q_sb[h][:, last_mc * 512 : (last_mc + 1) * 512], ps[:]
                    )
                    q3["ps"] = None

            def d_group(tt, eh):
                ps = ps_dg.tile([P, 1024], F32, tag="ps_dg")
                for h in range(HPC):
                    for es in range(2):
                        ec = 2 * eh + es
                        nc.tensor.matmul(
                            ps[:, es * 512 : (es + 1) * 512],
                            ctx_sb[h][:, tt * P : (tt + 1) * P],
                            wo_sb[h][:, ec * 512 : (ec + 1) * 512],
                            start=(h == 0),
                            stop=(h == HPC - 1),
                        )
                ob = opool.tile([P, 1024], F32, tag="ob")
                nc.vector.tensor_copy(ob[:], ps[:])
                nc.gpsimd.dma_start(
                    out[tt * P : (tt + 1) * P, eh * 1024 : (eh + 1) * 1024], ob[:]
                )

            def unit(jb, h, cap):
                """One (q-block, head): scoresT+exp, PV (2-pair lag), DVE
                denominator sums, deferred normalization; pulls up to `cap`
                interleaved PE work items (q3 quarters / out-proj groups)."""
                pulls = [0]

                def take():
                    if pulls[0] >= cap:
                        return
                    if q3["i"] < 4 * HPC:
                        q3_item()
                        pulls[0] += 1
                    elif dstate["taken"] < dstate["avail"]:
                        d_group(*DG[dstate["taken"]])
                        dstate["taken"] += 1
                        pulls[0] += 1

                qs = q_sb[h][:, jb * 512 : (jb + 1) * 512]
                pbt = pbt_pool.tile([P, NN, 512], BF16, tag="pbt", name="pbt")
                pbt_flat = pbt[:].rearrange("p n m -> p (n m)")
                psc = ps_cd.tile([P, 512], F32, tag="ps_cd")
                # wide-op denominator tree: A[q] = quarter partial [P,1024]
                aq = [atree.tile([P, 1024], BF16, tag=f"a{i}", name=f"a{i}")
                      for i in range(4)]

                def pv(nt):
                    nc.tensor.matmul(
                        psc[:], vvs[h][:, nt, :], pbt[:, nt, :],
                        start=(nt == 0), stop=(nt == NN - 1),
                    )

                for pair in range(8):
                    take()
                    ps = ps_s.tile([P, 1024], F32, tag="ps_s")
                    for sub in range(2):
                        nt = 2 * pair + sub
                        nc.tensor.matmul(
                            ps[:, sub * 512 : (sub + 1) * 512],
                            k_sb[h][:, nt * P : (nt + 1) * P],
                            qs,
                            start=True,
                            stop=True,
                        )
                    if with_mask:
                        mt = mpool.tile([P, 1024], F32, tag="mt")
                        for sub in range(2):
                            nt = 2 * pair + sub
                            nc.gpsimd.dma_start(
                                mt[:, sub * 512 : (sub + 1) * 512],
                                maskf[nt * P : (nt + 1) * P,
                                      jb * 512 : (jb + 1) * 512],
                            )
                        nc.vector.tensor_add(ps[:], ps[:], mt[:])
                    nc.scalar.activation(
                        pbt_flat[:, pair * 1024 : (pair + 1) * 1024],
                        ps[:],
                        mybir.ActivationFunctionType.Exp,
                        scale=SCALE,
                    )
                    # denominator partials on the DVE: after each odd pair a
                    # quarter (4 exp tiles) is complete -> one 1024-wide add
                    if pair % 2 == 1:
                        qq = pair // 2
                        nc.vector.tensor_add(
                            aq[qq][:],
                            pbt_flat[:, 4 * qq * 512 : (4 * qq + 2) * 512],
                            pbt_flat[:, (4 * qq + 2) * 512 : (4 * qq + 4) * 512],
                        )
                        if qq == 1:
                            nc.vector.tensor_add(aq[0][:], aq[0][:], aq[1][:])
                    # PV trails the exp by two pairs so it never waits on ACT
                    if pair >= 2:
                        pv(2 * (pair - 2))
                        pv(2 * (pair - 2) + 1)
                for nt in range(12, 16):
                    pv(nt)
                nc.vector.tensor_add(aq[2][:], aq[2][:], aq[3][:])
                nc.vector.tensor_add(aq[0][:], aq[0][:], aq[2][:])
                qfin = qf_pool.tile([P, 512], BF16, tag="qf", name="qfin")
                nc.vector.tensor_add(
                    qfin[:], aq[0][:, 0:512], aq[0][:, 512:1024]
                )

                ctmp = ctmp_pool.tile([P, 512], BF16, tag="ctmp", name="ctmp")
                nc.scalar.activation(
                    ctmp[:], psc[:], mybir.ActivationFunctionType.Copy
                )

                def finish():
                    # denominator: x-bar transpose -> free-axis reduce ->
                    # wide reciprocal
                    pst = pst_pool.tile([P, 4, P], BF16, tag="pst", name="pst")
                    nc.sync.dma_start(pst[:], qfin[:], transpose=True)
                    den = stats.tile([P, 4], F32, tag="den")
                    nc.vector.tensor_reduce(
                        den[:], pst[:], mybir.AxisListType.X, mybir.AluOpType.add
                    )
                    rect = stats.tile([P, 4], F32, tag="rect")
                    nc.vector.reciprocal(rect[:], den[:])
                    cn = cnrm_pool.tile([P, 4, P], BF16, tag="cnrm", name="cnrm")
                    nc.sync.dma_start(cn[:], ctmp[:], transpose=True)
                    for j in range(4):
                        nc.vector.tensor_scalar_mul(
                            cn[:, j, :], cn[:, j, :], rect[:, j : j + 1]
                        )
                    ctx_dst = ctx_sb[h][:, jb * 512 : (jb + 1) * 512].rearrange(
                        "p (a b) -> p a b", a=4
                    )
                    nc.sync.dma_start(ctx_dst, cn[:], transpose=True)

                return finish

            pending = []  # (finish, avail_bump)
            for jb in range(4):
                for h in range(HPC):
                    cap = 4 if jb == 0 else (3 if jb == 3 else 2)
                    fin = unit(jb, h, cap)
                    pending.append((fin, 8 if h == HPC - 1 else 0))
                    if len(pending) > 1:
                        f, bump = pending.pop(0)
                        f()
                        dstate["avail"] += bump
            for f, bump in pending:
                f()
                dstate["avail"] += bump
            while dstate["taken"] < len(DG):
                d_group(*DG[dstate["taken"]])
                dstate["taken"] += 1
            for tt in range(4 * (NM - 1), NQ):
                for eh in range(NE // 2):
                    d_group(tt, eh)


def _build(with_mask: bool):
    nc = bacc.Bacc("TRN2")
    xT = nc.dram_tensor("xT", [DIM, S], BF16, kind="ExternalInput")
    wqT = nc.dram_tensor("wqT", [DIM, DL], BF16, kind="ExternalInput")
    wkT = nc.dram_tensor("wkT", [DIM, DL], BF16, kind="ExternalInput")
    wvT = nc.dram_tensor("wvT", [DIM, DL], BF16, kind="ExternalInput")
    woT = nc.dram_tensor("woT", [DL, DIM], BF16, kind="ExternalInput")
    maskf = (
        nc.dram_tensor("maskf", [S, S], F32, kind="ExternalInput")
        if with_mask
        else None
    )
    out = nc.dram_tensor("out", [S, DIM], F32, kind="ExternalOutput")
    with tile.TileContext(nc) as tc:
        _emit(nc, tc, xT, wqT, wkT, wvT, woT, maskf, out)
    nc.finalize()
    return nc


def _get_program(with_mask: bool):
    if with_mask not in _PROGRAM_CACHE:
        _PROGRAM_CACHE[with_mask] = _build(with_mask)
    return _PROGRAM_CACHE[with_mask]


def _prep_in_maps(x, mask, wq, wk, wv, wo, with_mask):
    bf = ml_dtypes.bfloat16
    f32 = np.float32
    xTs = [np.ascontiguousarray(x[b].T.astype(bf)) for b in range(B)]
    if with_mask:
        maskf = np.ascontiguousarray(mask[0, 0].T.astype(f32) / SCALE)
    in_maps = []
    for c in range(N_CORES):
        b = c // HGROUPS
        g = c % HGROUPS
        sl = slice(g * DL, (g + 1) * DL)
        m = {
            "xT": xTs[b],
            "wqT": np.ascontiguousarray(wq[sl, :].T.astype(bf)),
            "wkT": np.ascontiguousarray(wk[sl, :].T.astype(bf)),
            "wvT": np.ascontiguousarray(wv[sl, :].T.astype(bf)),
            "woT": np.ascontiguousarray(wo[:, sl].T.astype(bf)),
        }
        if with_mask:
            m["maskf"] = maskf
        in_maps.append(m)
    return in_maps


def run_sharded(x, mask, wq, wk, wv, wo, trace=False, trace_kwargs=None):
    """Run the SPMD kernel; returns (full_output, BassKernelResults)."""
    with_mask = bool(np.any(np.asarray(mask)))
    nc = _get_program(with_mask)
    in_maps = _prep_in_maps(
        np.asarray(x), np.asarray(mask), np.asarray(wq), np.asarray(wk),
        np.asarray(wv), np.asarray(wo), with_mask,
    )
    kw = {}
    if trace:
        kw["trace"] = True
        if trace_kwargs:
            kw["trace_kwargs"] = trace_kwargs
    res = run_bass_kernel_spmd(nc, in_maps, list(range(N_CORES)), **kw)
    out = np.zeros((B, S, DIM), np.float32)
    for c in range(N_CORES):
        out[c // HGROUPS] += res.results[c]["out"]
    return out, res


def kernel(**inputs):
    out, _ = run_sharded(
        inputs["x"], inputs["mask"], inputs["wq"], inputs["wk"], inputs["wv"],
        inputs["wo"],
    )
    return out
